# revision 1
# baseline (speedup 1.0000x reference)
"""Trainium2 Bass kernel v2 for nn_MultiHeadAttention_86079734546451.

Sharding: data-parallel over batch B=16 across 8 cores (2 batches/core).
All weights replicated. No collectives.

v2 redesign vs baseline (1.59ms):
 - all inputs/weights host-cast to bf16 (halves input DMA bytes)
 - bias path: 4 batched abt DMAs per n-quarter; kron matmul with a
   strided rhs view so psum comes out (h, nb, mi)-ordered; contiguous
   cast-copies (spread over DVE/ACT/GpSimd) to bf16; DVE 32x32
   stream-transpose swaps s<->mi in one op per chunk. No xbar DMA
   transpose, no DRAM bounce.
 - scores/AV matmuls emitted head-pair-interleaved so the K=64 (scores)
   and M=64 (AV) matmuls pack two 64-row/col tiles of the PE array
   concurrently (tile_position auto-derived from base partitions).
 - batched exp over 4 s values per ACT op; softmax sums via DVE
   tensor_reduce; 1/z folded into v with a broadcast AP multiply.
 - bq and bd dropped (constant along the softmax axis -> cancel).
 - O-projection bias added via a K=1 ones-matmul; output DMAd straight
   from PSUM.
"""

import sys

sys.path.insert(0, "/opt/trn_rl_repo")

from contextlib import ExitStack

import numpy as np

import concourse.bass as bass
import concourse.mybir as mybir
import concourse.tile as tile
from concourse import bacc

f32 = mybir.dt.float32
bf16 = mybir.dt.bfloat16
AF = mybir.ActivationFunctionType
ALU = mybir.AluOpType
AX = mybir.AxisListType

# Problem constants
B_LOC = 2          # batches per core
D = 512
N = 128            # nodes
S = 14             # seq
L = 12
H = 8
DH = 64            # head dim
TOK = N * S        # 1792 tokens per batch, (n, s) order
C = 4              # 128-chunks of D
NCORES = 8

QUADS = [(0, 4), (4, 8), (8, 12), (12, 14)]  # s-blocks


def emit_kernel(ctx: ExitStack, tc: "tile.TileContext", io: dict):
    nc = tc.nc

    q_d, k_d, v_d, ab_d = io["q"], io["k"], io["v"], io["ab"]
    out_d = io["out"]

    # ---------------- pools ----------------
    wpool = ctx.enter_context(tc.tile_pool(name="wpool", bufs=1))
    xin = ctx.enter_context(tc.tile_pool(name="xin", bufs=4))
    qkh = ctx.enter_context(tc.tile_pool(name="qkh", bufs=8))
    vhp = ctx.enter_context(tc.tile_pool(name="vhp", bufs=1))
    tbp = ctx.enter_context(tc.tile_pool(name="tbp", bufs=1))
    abp = ctx.enter_context(tc.tile_pool(name="abp", bufs=1))
    osbp = ctx.enter_context(tc.tile_pool(name="osbp", bufs=2))
    bsbp = ctx.enter_context(tc.tile_pool(name="bsbp", bufs=2))
    ytp = ctx.enter_context(tc.tile_pool(name="ytp", bufs=4))
    ebp = ctx.enter_context(tc.tile_pool(name="ebp", bufs=4))
    zrp = ctx.enter_context(tc.tile_pool(name="zrp", bufs=2))
    vpp = ctx.enter_context(tc.tile_pool(name="vpp", bufs=2))

    pp = ctx.enter_context(tc.tile_pool(name="pp", bufs=2, space="PSUM"))
    pb = ctx.enter_context(tc.tile_pool(name="pb", bufs=2, space="PSUM"))
    scp = ctx.enter_context(tc.tile_pool(name="scp", bufs=2, space="PSUM"))

    # ---------------- weights (once) ----------------
    wq, wk, wv, wo = [], [], [], []
    for c in range(C):
        for dst, nm in ((wq, "wqT"), (wk, "wkT"), (wv, "wvT"), (wo, "woT")):
            t = wpool.tile([128, D], bf16, name=f"{nm}{c}", tag=f"{nm}{c}")
            nc.scalar.dma_start(t[:], io[nm][c * 128:(c + 1) * 128, :])
            dst.append(t)

    wdk = wpool.tile([48, 128], bf16, name="wdk", tag="wdk")
    nc.scalar.dma_start(wdk[:], io["wdk"][:])

    bk_t = wpool.tile([128, C], f32, name="bk_t", tag="bk_t")
    for c in range(C):
        nc.scalar.dma_start(bk_t[:, c:c + 1],
                            io["bk"][c * 128:(c + 1) * 128].unsqueeze(1))

    ones_b = wpool.tile([1, 128], bf16, name="ones_b", tag="ones_b")
    nc.vector.memset(ones_b[:], 1.0)
    bv_st = wpool.tile([1, D], bf16, name="bv_st", tag="bv_st")
    nc.gpsimd.dma_start(bv_st[:], io["bv"].unsqueeze(0))
    bo_st = wpool.tile([1, D], bf16, name="bo_st", tag="bo_st")
    nc.gpsimd.dma_start(bo_st[:], io["bo"].unsqueeze(0))

    # ---------------- per-batch body (generator: yields at unit
    # boundaries so the driver can interleave the two batches) --------
    def batch_work(b):
        # bias tb for this batch: [m, (h:8, s:32, ch:16, nb:8)] bf16
        tb = tbp.tile([128, 16 * H * 8 * 32], bf16, name="tb")

        cp_cnt = [0]
        bsb_cur = [None]

        # tb layout: (ch:16, hq:4, nb:8, h2:2, s:32)
        tbv = tb[:].rearrange("p (ch hq nb h2 s) -> p s ch hq h2 nb",
                              ch=16, hq=4, nb=8, h2=2, s=32)

        def kron_half(abt, quar, chl, t):
            """Bias projection for n-chunk ch = quar*4+chl, h-half t."""
            ch = quar * 4 + chl
            # abt free layout (n:32, h:8, mi:32), mi contiguous
            abtv = abt[:].rearrange("p (n h m) -> p n h m", n=32, h=H)
            pbt = pb.tile([128, 1024], f32, tag="pb", name="ps_b")
            for hq2 in range(2):
                hq = t * 2 + hq2
                rhs = abtv[:, chl * 8:(chl + 1) * 8,
                           hq * 2:(hq + 1) * 2, :]
                nc.tensor.matmul(
                    pbt[:, hq2 * 512:(hq2 + 1) * 512],
                    lhsT=wdk[:], rhs=rhs, start=True, stop=True)
            # cast-copy psum f32 -> bsb bf16 on ACT (frees psum fast,
            # decouples the stream-transpose from the PE/psum chain)
            if t == 0:
                bsb_cur[0] = bsbp.tile([128, 2048], bf16, tag="bsb",
                                       name="bsb")
            bsb = bsb_cur[0]
            nc.scalar.copy(bsb[:, t * 1024:(t + 1) * 1024], pbt[:])
            if t == 1:
                # 32x32 block transpose from SBUF:
                # [(j,s), (hq,nb,h2,mi)] -> [(j,mi), (hq,nb,h2,s)]
                tb_out = tb[:].rearrange(
                    "p (ch r) -> p ch r", ch=16)[:, ch, :]
                nc.vector.transpose(tb_out, bsb[:])

        def abt_load(quar):
            abt = abp.tile([48, 32 * 32 * H], bf16, tag="abt", name=f"abt{quar}")
            for j in range(4):
                nc.sync.dma_start(
                    abt[j * 12:(j + 1) * 12, :].rearrange(
                        "l (n h m) -> l n h m", n=32, h=H),
                    ab_d[b, :, j, quar * 32:(quar + 1) * 32, :, :])
            return abt

        def load_x(src_d):
            xs = []
            for ci in range(C):
                x_c = xin.tile([128, TOK], bf16, tag="xin", name=f"x{ci}")
                nc.gpsimd.dma_start(
                    x_c[:],
                    src_d[b, ci * 128:(ci + 1) * 128].rearrange(
                        "p n s -> p (n s)"))
                xs.append(x_c)
            return xs

        def p1_co(xs, wts, co, dst_list, with_bias):
            h_c = qkh.tile([128, TOK], bf16, tag="qkh", name=f"h{co}")
            for tbk in range(4):
                ps = pp.tile([128, 448], f32, tag="pp", name="ps_qk")
                for ci in range(C):
                    nc.tensor.matmul(
                        ps[:],
                        lhsT=wts[ci][:, co * 128:(co + 1) * 128],
                        rhs=xs[ci][:, tbk * 448:(tbk + 1) * 448],
                        start=(ci == 0), stop=(ci == C - 1))
                if with_bias:
                    nc.scalar.activation(h_c[:, tbk * 448:(tbk + 1) * 448],
                                         ps[:], AF.Identity,
                                         bias=bk_t[:, co:co + 1], scale=1.0)
                else:
                    nc.scalar.activation(h_c[:, tbk * 448:(tbk + 1) * 448],
                                         ps[:], AF.Identity, scale=1.0)
            dst_list.append(h_c)

        # ---- P1 Q/K/V + P2 interleaved (kron halves spread out so the
        # in-order PE queue never head-of-line blocks on the psum chain)
        qh, kh = [], []
        yield "p1"
        xq = load_x(q_d)
        abt0 = abt_load(0)
        for co in range(C):
            if co:
                yield "p1"
            p1_co(xq, wq, co, qh, False)
            kron_half(abt0, 0, co, 0)
            kron_half(abt0, 0, co, 1)

        yield "p1"
        xk = load_x(k_d)
        abt1 = abt_load(1)
        for co in range(C):
            if co:
                yield "p1"
            p1_co(xk, wk, co, kh, True)
            kron_half(abt1, 1, co, 0)
            kron_half(abt1, 1, co, 1)

        # V projection -> vh [m, (s, h, d)] bf16 (+bv via ones-matmul)
        yield "v"
        xv = load_x(v_d)
        abt2 = abt_load(2)
        vh = vhp.tile([128, S * D], bf16, name="vh")
        abt3 = None
        for s in range(S):
            if s:
                yield "v"
            ps = pp.tile([128, D], f32, tag="pp", name="ps_v")
            for ci in range(C):
                nc.tensor.matmul(
                    ps[:],
                    lhsT=xv[ci][:, s::S],
                    rhs=wv[ci][:],
                    start=(ci == 0), stop=False)
            nc.tensor.matmul(ps[:], lhsT=ones_b[:], rhs=bv_st[:],
                             start=False, stop=True)
            nc.vector.tensor_copy(vh[:, s * D:(s + 1) * D], ps[:])
            if s < 4:
                kron_half(abt2, 2, s, 0)
                kron_half(abt2, 2, s, 1)
            elif s == 4:
                abt3 = abt_load(3)
            elif 5 <= s < 9:
                kron_half(abt3, 3, s - 5, 0)
                kron_half(abt3, 3, s - 5, 1)

        # ---- P3 attention per head-pair, software-pipelined:
        # scores of hp+1 interleave with AV of hp so the PE stays busy
        # during the exp/z/recip tail of each head-pair.
        yield "p3"
        z_t = zrp.tile([128, 16 * H], f32, tag="z", name="z_t")
        rt_t = zrp.tile([128, 16 * H], f32, tag="r", name="rt_t")
        zv = z_t[:].rearrange("p (h s) -> p h s", h=H)
        rvT = rt_t[:].rearrange("p (h s) -> p s h", h=H)
        rv = rt_t[:].rearrange("p (h s) -> p h s", h=H)

        yt = [ytp.tile([128, TOK], bf16, tag="ytp", name=f"yt{c}")
              for c in range(C)]
        ebts = {}
        vhv = vh[:].rearrange("p (s d) -> p s d", s=S)

        def start_a(hp):
            ebts[hp] = [
                ebp.tile([128, S * 128], bf16, tag="ebt", name=f"eb{hp}_{h2}")
                for h2 in range(2)]

        def a_quad(hp, qi):
            c = hp
            s0, s1 = QUADS[qi]
            ls = s1 - s0
            sct = [scp.tile([128, ls * 128], f32, tag="sc", name="sc_t")
                   for _ in range(2)]
            for si in range(ls):
                s = s0 + si
                for h2 in range(2):
                    hb = h2 * DH
                    nc.tensor.matmul(
                        sct[h2][:, si * 128:(si + 1) * 128],
                        lhsT=kh[c][hb:hb + DH, s::S],
                        rhs=qh[c][hb:hb + DH, s::S],
                        start=True, stop=True)
            for h2 in range(2):
                h = hp * 2 + h2
                scv = sct[h2][:].rearrange(
                    "p (s ch nb) -> p s ch nb", s=ls, ch=16)
                ebv = ebts[hp][h2][:, s0 * 128:s1 * 128].rearrange(
                    "p (s ch nb) -> p s ch nb", s=ls, ch=16)
                biasv = tbv[:, s0:s1, :, h // 2, h % 2, :]
                nc.vector.tensor_add(ebv, scv, biasv)

        def a_tail(hp):
            ebt = ebts[hp]
            for h2 in range(2):
                h = hp * 2 + h2
                nc.scalar.activation(ebt[h2][:], ebt[h2][:], AF.Exp)
                nc.vector.tensor_reduce(
                    zv[:, h, :S],
                    ebt[h2][:].rearrange("p (s n) -> p s n", s=S),
                    AX.X, ALU.add)
            nc.vector.reciprocal(rv[:, 2 * hp:2 * hp + 2, :S],
                                 zv[:, 2 * hp:2 * hp + 2, :S])

        def b_quad(hp, qi):
            c = hp
            ebt = ebts[hp]
            s0, s1 = QUADS[qi]
            ls = s1 - s0
            vpt = vpp.tile([128, ls * 128], bf16, tag="vp", name="vp_t")
            vpv = vpt[:].rearrange("p (s h2 dd) -> p s h2 dd",
                                   s=ls, h2=2)
            src = vhv[:, s0:s1, hp * 128:(hp + 1) * 128].rearrange(
                "p s (h2 dd) -> p s h2 dd", h2=2)
            rtb = rvT[:, s0:s1, 2 * hp:2 * hp + 2].unsqueeze(
                3).broadcast_to([128, ls, 2, DH])
            nc.gpsimd.tensor_mul(vpv, src, rtb)
            av = pp.tile([128, ls * 128], f32, tag="pp", name="av_t")
            for si in range(ls):
                s = s0 + si
                for h2 in range(2):
                    nc.tensor.matmul(
                        av[h2 * DH:(h2 + 1) * DH,
                           si * 128:(si + 1) * 128],
                        lhsT=vpv[:, si, h2, :],
                        rhs=ebt[h2][:, s * 128:(s + 1) * 128],
                        start=True, stop=True)
            nc.vector.tensor_copy(yt[c][:, s0 * 128:s1 * 128], av[:])

        start_a(0)
        for qi in range(4):
            a_quad(0, qi)
        for hp in range(C):
            yield "p3"
            a_tail(hp)
            if hp < C - 1:
                start_a(hp + 1)
                for qi in range(4):
                    a_quad(hp + 1, qi)
                    b_quad(hp, qi)
            else:
                for qi in range(4):
                    b_quad(hp, qi)
            del ebts[hp]

        # ---- P4 output projection, bias via ones-matmul
        for s in range(S):
            yield "p4"
            ps = pp.tile([128, D], f32, tag="pp", name="ps_o")
            for ci in range(C):
                nc.tensor.matmul(
                    ps[:],
                    lhsT=yt[ci][:, s * 128:(s + 1) * 128],
                    rhs=wo[ci][:],
                    start=(ci == 0), stop=False)
            nc.tensor.matmul(ps[:], lhsT=ones_b[:], rhs=bo_st[:],
                             start=False, stop=True)
            osb = osbp.tile([128, D], f32, tag="osb", name="osb")
            nc.scalar.copy(osb[:], ps[:])
            eng = nc.sync if s % 2 == 0 else nc.scalar
            eng.dma_start(out_d[b, s], osb[:])

    # ---------------- driver: overlap P4(b0) with P1qk(b1) ----------
    g0, g1 = batch_work(0), batch_work(1)
    t0 = next(g0)
    while t0 != "p4":
        t0 = next(g0)
    t1 = next(g1)
    while t0 != "done":
        t0 = next(g0, "done")
        if t1 == "p1":
            t1 = next(g1)
    while t1 != "done":
        t1 = next(g1, "done")


def build_nc():
    nc = bacc.Bacc("TRN2", target_bir_lowering=False, debug=False,
                   num_devices=NCORES)
    io = {}
    io["q"] = nc.dram_tensor("q", [B_LOC, D, N, S], bf16, kind="ExternalInput").ap()
    io["k"] = nc.dram_tensor("k", [B_LOC, D, N, S], bf16, kind="ExternalInput").ap()
    io["v"] = nc.dram_tensor("v", [B_LOC, D, N, S], bf16, kind="ExternalInput").ap()
    # ab host-transposed to [b, l, j, n, h, mi] (m = j*32 + mi) so the
    # abt DMA gets 512B runs, the kron rhs is mi-contiguous, and the
    # stream-transpose sees mi as the inner 32-block
    io["ab"] = nc.dram_tensor("ab", [B_LOC, L, 4, N, H, 32], bf16,
                              kind="ExternalInput").ap()
    for nm in ("wqT", "wkT", "wvT", "woT"):
        io[nm] = nc.dram_tensor(nm, [D, D], bf16, kind="ExternalInput").ap()
    io["bk"] = nc.dram_tensor("bk", [D], f32, kind="ExternalInput").ap()
    io["bv"] = nc.dram_tensor("bv", [D], f32, kind="ExternalInput").ap()
    io["bo"] = nc.dram_tensor("bo", [D], f32, kind="ExternalInput").ap()
    io["wdk"] = nc.dram_tensor("wdk", [48, 128], bf16, kind="ExternalInput").ap()
    io["out"] = nc.dram_tensor("out", [B_LOC, S, N, D], f32,
                               kind="ExternalOutput").ap()

    with tile.TileContext(nc) as tc:
        with ExitStack() as ctx:
            emit_kernel(ctx, tc, io)
    nc.compile()
    return nc


def host_prep(Wq, bq, Wk, bk, Wv, bv, Wd, bd, Wo, bo):
    """Pre-transpose weights to bf16; fold qk scale into Wq; drop bq/bd
    (constant along the softmax axis); build kron weight with rows
    (j*12+l) and cols (j*32+s)."""
    import ml_dtypes
    b16 = ml_dtypes.bfloat16
    scale = (D // H) ** -0.5
    prep = {
        "wqT": np.ascontiguousarray((Wq * scale).T).astype(b16),
        "wkT": np.ascontiguousarray(Wk.T).astype(b16),
        "wvT": np.ascontiguousarray(Wv.T).astype(b16),
        "woT": np.ascontiguousarray(Wo.T).astype(b16),
        "bk": np.asarray(bk, np.float32),
        "bv": np.asarray(bv, np.float32),
        "bo": np.asarray(bo, np.float32),
    }
    wdk = np.zeros((48, 128), np.float32)
    for j in range(4):
        wdk[j * 12 + np.arange(L), j * 32:j * 32 + S] = np.asarray(Wd).T
    prep["wdk"] = wdk.astype(b16)
    return prep


_NC_CACHE = None


def run(q, k, v, attn_bias, Wq, bq, Wk, bk, Wv, bv, Wd, bd, Wo, bo,
        trace=False, **trace_kwargs):
    global _NC_CACHE
    import ml_dtypes
    from concourse.bass_utils import run_bass_kernel_spmd

    b16 = ml_dtypes.bfloat16
    if _NC_CACHE is None:
        _NC_CACHE = build_nc()
    nc = _NC_CACHE

    prep = host_prep(Wq, bq, Wk, bk, Wv, bv, Wd, bd, Wo, bo)
    q = np.asarray(q).astype(b16)
    k = np.asarray(k).astype(b16)
    v = np.asarray(v).astype(b16)
    B = np.asarray(attn_bias).shape[0]
    # [B, L, n, m, H] -> [B, L, j, n, H, mi] with m = j*32 + mi
    ab = np.asarray(attn_bias).reshape(B, L, N, 4, 32, H).transpose(
        0, 1, 3, 2, 5, 4).astype(b16)

    in_maps = []
    for i in range(NCORES):
        sl = slice(i * B_LOC, (i + 1) * B_LOC)
        in_maps.append({
            "q": np.ascontiguousarray(q[sl]),
            "k": np.ascontiguousarray(k[sl]),
            "v": np.ascontiguousarray(v[sl]),
            "ab": np.ascontiguousarray(ab[sl]),
            **prep,
        })
    res = run_bass_kernel_spmd(nc, in_maps, list(range(NCORES)), trace=trace,
                               **trace_kwargs)
    out = np.concatenate([res.results[i]["out"] for i in range(NCORES)], axis=0)
    return out, res


def kernel(**inputs):
    return run(**inputs)[0]



# revision 12
# speedup vs baseline: 1.2833x; 1.2833x over previous
"""Trainium2 Bass kernel v3 for nn_MultiHeadAttention_86079734546451.

Sharding: data-parallel over batch B=16 across 8 cores (2 batches/core).
All weights replicated. No collectives.

v3 redesign vs v2 (417us):
 - q/k/v host-transposed to [B, D, S, N] (s-major tokens): every per-s
   slice in scores/V-proj/AV is contiguous (v2 paid ~3x on strided
   s::S LDWEIGHTS/rhs streaming).
 - bias kron packs BOTH batches in one matmul: contraction (j,b2,l)=96
   partitions, block-diag wdk2 outputs (j,b2,s:16) partitions. Halves
   kron MMs, psum-evac copies, and transposes.
 - psum->sbuf kron copy IS the exp: softmax factorized as
   exp(scores+bias) = exp(scores)*exp(bias); ebias transposed to
   [m, (ch,hq,nb,b2,s,h2)] via DVE 32x32 stream transpose done on an
   int32 view (h2-pair rides innermost) -> half the transpose elems.
 - scores exp: ACT reads psum directly per s-quad; the bias multiply is
   a DVE scalar_tensor_tensor per (s,h2) with accum_out -> softmax z
   falls out free (no tensor_reduce, no psum tensor_add).
 - bv folded into the V psum evacuation (DVE tensor_add with a
   partition-replicated bv tile); output staged bf16.
 - driver interleaves kron||P1(b0), P3(b0)||P1(b1), P4(b0)||P3(b1).
"""

import sys

sys.path.insert(0, "/opt/trn_rl_repo")

from contextlib import ExitStack

import numpy as np

import concourse.bass as bass
import concourse.mybir as mybir
import concourse.tile as tile
from concourse import bacc

f32 = mybir.dt.float32
bf16 = mybir.dt.bfloat16
i32 = mybir.dt.int32
AF = mybir.ActivationFunctionType
ALU = mybir.AluOpType
AX = mybir.AxisListType

# Problem constants
B_LOC = 2          # batches per core
D = 512
N = 128            # nodes
S = 14             # seq
L = 12
H = 8
DH = 64            # head dim
TOK = N * S        # 1792 tokens per batch, (s, n) order
C = 4              # 128-chunks of D
NCORES = 8

QUADS = [(0, 4), (4, 8), (8, 12), (12, 14)]  # s-blocks


def emit_kernel(ctx: ExitStack, tc: "tile.TileContext", io: dict):
    nc = tc.nc

    q_d, k_d, v_d, ab_d = io["q"], io["k"], io["v"], io["ab"]
    out_d = io["out"]

    # ---------------- pools ----------------
    wpool = ctx.enter_context(tc.tile_pool(name="wpool", bufs=1))
    tbp = ctx.enter_context(tc.tile_pool(name="tbp", bufs=1))
    bsbp = ctx.enter_context(tc.tile_pool(name="bsbp", bufs=2))
    abp = ctx.enter_context(tc.tile_pool(name="abp", bufs=1))
    xin = ctx.enter_context(tc.tile_pool(name="xin", bufs=8))
    qkh = ctx.enter_context(tc.tile_pool(name="qkh", bufs=1))
    vhp = ctx.enter_context(tc.tile_pool(name="vhp", bufs=2))
    ebp = ctx.enter_context(tc.tile_pool(name="ebp", bufs=3))
    ytp = ctx.enter_context(tc.tile_pool(name="ytp", bufs=4))
    vpp = ctx.enter_context(tc.tile_pool(name="vpp", bufs=2))
    zrp = ctx.enter_context(tc.tile_pool(name="zrp", bufs=1))
    osbp = ctx.enter_context(tc.tile_pool(name="osbp", bufs=2))

    pb = ctx.enter_context(tc.tile_pool(name="pb", bufs=2, space="PSUM"))
    pq = ctx.enter_context(tc.tile_pool(name="pq", bufs=2, space="PSUM"))
    scp = ctx.enter_context(tc.tile_pool(name="scp", bufs=2, space="PSUM"))

    # ---------------- weights (once) ----------------
    wq, wk, wv, wo = [], [], [], []
    for c in range(C):
        for dst, nm in ((wq, "wqT"), (wk, "wkT"), (wv, "wvT"), (wo, "woT")):
            t = wpool.tile([128, D], bf16, name=f"{nm}{c}", tag=f"{nm}{c}")
            nc.scalar.dma_start(t[:], io[nm][c * 128:(c + 1) * 128, :])
            dst.append(t)

    wdk2 = wpool.tile([96, 128], bf16, name="wdk2", tag="wdk2")
    nc.scalar.dma_start(wdk2[:], io["wdk2"][:])

    bk_t = wpool.tile([128, C], f32, name="bk_t", tag="bk_t")
    for c in range(C):
        nc.scalar.dma_start(bk_t[:, c:c + 1],
                            io["bk"][c * 128:(c + 1) * 128].unsqueeze(1))

    ones_b = wpool.tile([1, 128], bf16, name="ones_b", tag="ones_b")
    nc.vector.memset(ones_b[:], 1.0)
    bo_st = wpool.tile([1, D], bf16, name="bo_st", tag="bo_st")
    nc.gpsimd.dma_start(bo_st[:], io["bo"].unsqueeze(0))
    bv_t = wpool.tile([128, D], bf16, name="bv_t", tag="bv_t")
    nc.gpsimd.dma_start(bv_t[:], io["bv2"][:])

    # ---------------- shared bias tile (both batches) ----------------
    # tb free layout: (ch:16, hq:4, nb:8, b2:2, s:16, h2:2), partition m
    tb = tbp.tile([128, 16 * 2048], bf16, name="tb", tag="tb")
    tb32v = tb[:].bitcast(i32).rearrange("p (ch r) -> p ch r", ch=16)
    tbv = tb[:].rearrange(
        "p (ch hq nb b2 s h2) -> p b2 hq h2 s ch nb",
        ch=16, hq=4, nb=8, b2=2, s=16, h2=2)

    def kron_gen():
        """ebias = exp(Wd-projected attn_bias), transposed to m-partition
        layout, both batches at once."""
        ab_v = ab_d[:].rearrange("q p (n r) -> q p n r", n=32)
        for quar in range(4):
            for half in range(2):
                abt = abp.tile([96, 4096], bf16, tag="abt", name="abt")
                nc.sync.dma_start(
                    abt[:].rearrange("p (n r) -> p n r", n=16),
                    ab_v[quar, :, half * 16:(half + 1) * 16, :])
                abtv = abt[:].rearrange(
                    "p (n hq mi h2) -> p n hq mi h2", n=16, hq=4, mi=32)
                for chl in range(2):
                    ch = quar * 4 + half * 2 + chl
                    bsb = bsbp.tile([128, 2048], bf16, tag="bsb", name="bsb")
                    for t in range(2):
                        pbt = pb.tile([128, 1024], f32, tag="pb", name="pbt")
                        for hq2 in range(2):
                            hq = t * 2 + hq2
                            nc.tensor.matmul(
                                pbt[:, hq2 * 512:(hq2 + 1) * 512],
                                lhsT=wdk2[:],
                                rhs=abtv[:, chl * 8:(chl + 1) * 8, hq, :, :],
                                start=True, stop=True)
                        nc.scalar.activation(
                            bsb[:, t * 1024:(t + 1) * 1024], pbt[:], AF.Exp)
                        yield "kron"
                    nc.vector.transpose(tb32v[:, ch, :], bsb[:].bitcast(i32))
                    yield "kron"

    # ---------------- per-batch body ----------------
    def batch_work(b):
        def load_x(src_d):
            xs = []
            for ci in range(C):
                x_c = xin.tile([128, TOK], bf16, tag="xin", name=f"x{ci}")
                nc.gpsimd.dma_start(
                    x_c[:],
                    src_d[b, ci * 128:(ci + 1) * 128].rearrange(
                        "p s n -> p (s n)"))
                xs.append(x_c)
            return xs

        def p1_co(xs, wts, co, dst_list, with_bias, tg):
            # per-name tag, bufs=1: b1's gen reuses exactly b0's slot,
            # which frees when b0's P3 hp=co scores have read it
            h_c = qkh.tile([128, TOK], bf16, tag=f"h{tg}{co}", name=f"h{tg}{co}")
            for tbk in range(4):
                ps = pq.tile([128, 512], f32, tag="pq", name="ps_qk")
                for ci in range(C):
                    nc.tensor.matmul(
                        ps[:, :448],
                        lhsT=wts[ci][:, co * 128:(co + 1) * 128],
                        rhs=xs[ci][:, tbk * 448:(tbk + 1) * 448],
                        start=(ci == 0), stop=(ci == C - 1))
                if with_bias:
                    nc.scalar.activation(h_c[:, tbk * 448:(tbk + 1) * 448],
                                         ps[:, :448], AF.Identity,
                                         bias=bk_t[:, co:co + 1], scale=1.0)
                else:
                    nc.scalar.activation(h_c[:, tbk * 448:(tbk + 1) * 448],
                                         ps[:, :448], AF.Identity, scale=1.0)
            dst_list.append(h_c)

        # ---- P1 ----
        qh, kh = [], []
        yield "p1"
        xq = load_x(q_d)
        xk = load_x(k_d)
        for co in range(C):
            # gate marker: for b1 the driver holds here until b0's P3
            # hp=co scores are emitted (qkh slot release order)
            yield f"p1g{co}"
            p1_co(xq, wq, co, qh, False, "q")
            p1_co(xk, wk, co, kh, True, "k")
        yield "p1"
        xv = load_x(v_d)
        vh = vhp.tile([128, S * D], bf16, tag="vh", name="vh")
        for s in range(S):
            ps = pq.tile([128, D], f32, tag="pq", name="ps_v")
            for ci in range(C):
                nc.tensor.matmul(
                    ps[:],
                    lhsT=xv[ci][:, s * N:(s + 1) * N],
                    rhs=wv[ci][:],
                    start=(ci == 0), stop=(ci == C - 1))
            nc.vector.tensor_add(vh[:, s * D:(s + 1) * D], ps[:], bv_t[:])
            if s % 2:
                yield "p1"

        # ---- P3 ----
        z_t = zrp.tile([128, 128], f32, tag="z", name="z_t")
        rt_t = zrp.tile([128, 128], f32, tag="r", name="rt_t")
        zv = z_t[:].rearrange("p (h s) -> p h s", h=H)
        rv = rt_t[:].rearrange("p (h s) -> p h s", h=H)
        rvT = rt_t[:].rearrange("p (h s) -> p s h", h=H)
        vhv = vh[:].rearrange("p (s d) -> p s d", s=S)
        yt = [ytp.tile([128, TOK], bf16, tag="ytp", name=f"yt{c}")
              for c in range(C)]

        pend = [None]  # deferred AV work (one-quad software pipeline)

        def do_av(hp, qi, vpv, ex):
            s0, s1 = QUADS[qi]
            ls = s1 - s0
            av = pb.tile([128, 512], f32, tag="pb", name="av_t")
            for si in range(ls):
                for h2 in range(2):
                    nc.tensor.matmul(
                        av[h2 * DH:(h2 + 1) * DH, si * 128:(si + 1) * 128],
                        lhsT=vpv[:, si, h2, :],
                        rhs=ex[h2][:, si * 128:(si + 1) * 128],
                        start=True, stop=True)
            nc.vector.tensor_copy(yt[hp][:, s0 * 128:s1 * 128],
                                  av[:, :ls * 128])

        for hp in range(C):
            for qi, (s0, s1) in enumerate(QUADS):
                ls = s1 - s0
                # scores for this quad
                sct = [scp.tile([128, 512], f32, tag="sc", name=f"sc{h2}")
                       for h2 in range(2)]
                for si in range(ls):
                    s = s0 + si
                    for h2 in range(2):
                        hb = h2 * DH
                        nc.tensor.matmul(
                            sct[h2][:, si * 128:(si + 1) * 128],
                            lhsT=kh[hp][hb:hb + DH, s * N:(s + 1) * N],
                            rhs=qh[hp][hb:hb + DH, s * N:(s + 1) * N],
                            start=True, stop=True)
                yield "p3"
                # deferred AV of previous quad (PE work lands after the
                # interleaved P1 units gave the chain time to finish)
                if pend[0] is not None:
                    do_av(*pend[0])
                    pend[0] = None
                # exp(scores) from psum; bias-mul with accum -> z
                ex = [ebp.tile([128, 512], bf16, tag="eb", name=f"eb{h2}")
                      for h2 in range(2)]
                for h2 in range(2):
                    nc.scalar.activation(ex[h2][:, :ls * 128],
                                         sct[h2][:, :ls * 128], AF.Exp)
                for h2 in range(2):
                    h = hp * 2 + h2
                    for si in range(ls):
                        s = s0 + si
                        esl = ex[h2][:, si * 128:(si + 1) * 128].rearrange(
                            "p (ch nb) -> p ch nb", ch=16)
                        nc.vector.scalar_tensor_tensor(
                            esl, esl, 1.0, tbv[:, b, hp, h2, s],
                            op0=ALU.mult, op1=ALU.mult,
                            accum_out=zv[:, h, s:s + 1])
                nc.vector.reciprocal(rv[:, 2 * hp:2 * hp + 2, s0:s1],
                                     zv[:, 2 * hp:2 * hp + 2, s0:s1])
                # vp = vh * (1/z), broadcast over d
                vpt = vpp.tile([128, 512], bf16, tag="vp", name="vp_t")
                vpv = vpt[:, :ls * 128].rearrange(
                    "p (s h2 dd) -> p s h2 dd", s=ls, h2=2)
                src = vhv[:, s0:s1, hp * 128:(hp + 1) * 128].rearrange(
                    "p s (h2 dd) -> p s h2 dd", h2=2)
                rtb = rvT[:, s0:s1, 2 * hp:2 * hp + 2].unsqueeze(
                    3).broadcast_to([128, ls, 2, DH])
                nc.gpsimd.tensor_mul(vpv, src, rtb)
                pend[0] = (hp, qi, vpv, ex)
                yield "p3"
        do_av(*pend[0])
        pend[0] = None

        # ---- P4 ----
        for s in range(S):
            yield "p4"
            ps = pq.tile([128, D], f32, tag="pq", name="ps_o")
            for ci in range(C):
                nc.tensor.matmul(
                    ps[:],
                    lhsT=yt[ci][:, s * 128:(s + 1) * 128],
                    rhs=wo[ci][:],
                    start=(ci == 0), stop=False)
            nc.tensor.matmul(ps[:], lhsT=ones_b[:], rhs=bo_st[:],
                             start=False, stop=True)
            osb = osbp.tile([128, D], bf16, tag="osb", name="osb")
            nc.scalar.copy(osb[:], ps[:])
            eng = nc.sync if s % 2 == 0 else nc.scalar
            eng.dma_start(out_d[b, s], osb[:])

    # ---------------- driver ----------------
    # kron || P1(b0); kron fully drained before P3(b0)'s first stt hits
    # the DVE queue (its reads of tb must queue after all transposes);
    # then P3(b0) || P1(b1) with b1's QK-co_k gated on b0's hp_k scores
    # (qkh slot release order keeps the engine queues acyclic);
    # P3(b1)/P4(b1) emitted only after g0 is fully emitted.
    gk, g0, g1 = kron_gen(), batch_work(0), batch_work(1)
    t0 = next(g0)
    t1 = None
    g1_started = False
    kron_alive = True
    p3_cnt = 0

    def is_p1(t):
        return t is not None and t.startswith("p1")

    while t0 is not None or t1 is not None:
        if t0 is not None:
            t0 = next(g0, None)
            if t0 == "p3":
                p3_cnt += 1
        if kron_alive:
            # ~4 kron units per P1(b0) unit; full drain once g0 leaves P1
            n = 4 if is_p1(t0) else 10 ** 9
            for _ in range(n):
                if next(gk, None) is None:
                    kron_alive = False
                    break
        if not g1_started:
            if t0 is None or not is_p1(t0):
                g1_started = True
                t1 = next(g1, None)
        elif t1 is not None:
            if t0 is None:
                # g0 fully emitted: drain g1
                while t1 is not None:
                    t1 = next(g1, None)
            elif is_p1(t1):
                # advance g1 through its P1 only; hold at a gate marker
                # p1g{k} until b0's hp_k scores are emitted (8k+7 p3
                # yields); g1's P3 waits for g0 to finish entirely
                for _ in range(2):
                    if t1.startswith("p1g"):
                        k = int(t1[3:])
                        if p3_cnt < 8 * k + 7:
                            break
                    t1 = next(g1, None)
                    if t1 is None or not is_p1(t1):
                        break


def build_nc():
    nc = bacc.Bacc("TRN2", target_bir_lowering=False, debug=False,
                   num_devices=NCORES)
    io = {}
    # s-major tokens: [b, d, s, n]
    io["q"] = nc.dram_tensor("q", [B_LOC, D, S, N], bf16, kind="ExternalInput").ap()
    io["k"] = nc.dram_tensor("k", [B_LOC, D, S, N], bf16, kind="ExternalInput").ap()
    io["v"] = nc.dram_tensor("v", [B_LOC, D, S, N], bf16, kind="ExternalInput").ap()
    # ab host layout: [quar, (j b2 l)=96, (n32 hq mi h2)=8192]
    io["ab"] = nc.dram_tensor("ab", [4, 96, 8192], bf16,
                              kind="ExternalInput").ap()
    for nm in ("wqT", "wkT", "wvT", "woT"):
        io[nm] = nc.dram_tensor(nm, [D, D], bf16, kind="ExternalInput").ap()
    io["bk"] = nc.dram_tensor("bk", [D], f32, kind="ExternalInput").ap()
    io["bv2"] = nc.dram_tensor("bv2", [128, D], bf16, kind="ExternalInput").ap()
    io["bo"] = nc.dram_tensor("bo", [D], bf16, kind="ExternalInput").ap()
    io["wdk2"] = nc.dram_tensor("wdk2", [96, 128], bf16, kind="ExternalInput").ap()
    io["out"] = nc.dram_tensor("out", [B_LOC, S, N, D], bf16,
                               kind="ExternalOutput").ap()

    with tile.TileContext(nc) as tc:
        with ExitStack() as ctx:
            emit_kernel(ctx, tc, io)
    nc.compile()
    return nc


def host_prep(Wq, bq, Wk, bk, Wv, bv, Wd, bd, Wo, bo):
    """Pre-transpose weights to bf16; fold qk scale into Wq; drop bq/bd
    (constant along the softmax axis); build the two-batch kron weight."""
    import ml_dtypes
    b16 = ml_dtypes.bfloat16
    scale = (D // H) ** -0.5
    prep = {
        "wqT": np.ascontiguousarray((Wq * scale).T).astype(b16),
        "wkT": np.ascontiguousarray(Wk.T).astype(b16),
        "wvT": np.ascontiguousarray(Wv.T).astype(b16),
        "woT": np.ascontiguousarray(Wo.T).astype(b16),
        "bk": np.asarray(bk, np.float32),
        "bv2": np.ascontiguousarray(
            np.broadcast_to(np.asarray(bv, np.float32), (128, D))).astype(b16),
        "bo": np.asarray(bo).astype(b16),
    }
    wdk2 = np.zeros((96, 128), np.float32)
    WdT = np.asarray(Wd).T  # [L, S]
    for j in range(4):
        for b2 in range(2):
            r0, c0 = j * 24 + b2 * 12, j * 32 + b2 * 16
            wdk2[r0:r0 + L, c0:c0 + S] = WdT
    prep["wdk2"] = wdk2.astype(b16)
    return prep


_NC_CACHE = None


def run(q, k, v, attn_bias, Wq, bq, Wk, bk, Wv, bv, Wd, bd, Wo, bo,
        trace=False, **trace_kwargs):
    global _NC_CACHE
    import ml_dtypes
    from concourse.bass_utils import run_bass_kernel_spmd

    b16 = ml_dtypes.bfloat16
    if _NC_CACHE is None:
        _NC_CACHE = build_nc()
    nc = _NC_CACHE

    prep = host_prep(Wq, bq, Wk, bk, Wv, bv, Wd, bd, Wo, bo)
    # [B, D, N, S] -> [B, D, S, N]
    q = np.asarray(q).transpose(0, 1, 3, 2).astype(b16)
    k = np.asarray(k).transpose(0, 1, 3, 2).astype(b16)
    v = np.asarray(v).transpose(0, 1, 3, 2).astype(b16)
    ab = np.asarray(attn_bias)
    B = ab.shape[0]

    in_maps = []
    for i in range(NCORES):
        sl = slice(i * B_LOC, (i + 1) * B_LOC)
        # per-core ab: [b2, l, (quar n32), (j mi), (hq h2)] ->
        # [quar, j, b2, l, n32, hq, mi, h2]
        abc = ab[sl].reshape(2, L, 4, 32, 4, 32, 4, 2)
        abc = abc.transpose(2, 4, 0, 1, 3, 6, 5, 7)
        abc = np.ascontiguousarray(abc).astype(b16).reshape(4, 96, 8192)
        in_maps.append({
            "q": np.ascontiguousarray(q[sl]),
            "k": np.ascontiguousarray(k[sl]),
            "v": np.ascontiguousarray(v[sl]),
            "ab": abc,
            **prep,
        })
    res = run_bass_kernel_spmd(nc, in_maps, list(range(NCORES)), trace=trace,
                               **trace_kwargs)
    out = np.concatenate(
        [res.results[i]["out"].astype(np.float32) for i in range(NCORES)],
        axis=0)
    return out, res


def kernel(**inputs):
    return run(**inputs)[0]


# revision 18
# speedup vs baseline: 1.3275x; 1.0345x over previous
"""Trainium2 Bass kernel v3 for nn_MultiHeadAttention_86079734546451.

Sharding: data-parallel over batch B=16 across 8 cores (2 batches/core).
All weights replicated. No collectives.

v3 redesign vs v2 (417us):
 - q/k/v host-transposed to [B, D, S, N] (s-major tokens): every per-s
   slice in scores/V-proj/AV is contiguous (v2 paid ~3x on strided
   s::S LDWEIGHTS/rhs streaming).
 - bias kron packs BOTH batches in one matmul: contraction (j,b2,l)=96
   partitions, block-diag wdk2 outputs (j,b2,s:16) partitions. Halves
   kron MMs, psum-evac copies, and transposes.
 - psum->sbuf kron copy IS the exp: softmax factorized as
   exp(scores+bias) = exp(scores)*exp(bias); ebias transposed to
   [m, (ch,hq,nb,b2,s,h2)] via DVE 32x32 stream transpose done on an
   int32 view (h2-pair rides innermost) -> half the transpose elems.
 - scores exp: ACT reads psum directly per s-quad; the bias multiply is
   a DVE scalar_tensor_tensor per (s,h2) with accum_out -> softmax z
   falls out free (no tensor_reduce, no psum tensor_add).
 - bv folded into the V psum evacuation (DVE tensor_add with a
   partition-replicated bv tile); output staged bf16.
 - driver interleaves kron||P1(b0), P3(b0)||P1(b1), P4(b0)||P3(b1).
"""

import sys

sys.path.insert(0, "/opt/trn_rl_repo")

from contextlib import ExitStack

import numpy as np

import concourse.bass as bass
import concourse.mybir as mybir
import concourse.tile as tile
from concourse import bacc

f32 = mybir.dt.float32
bf16 = mybir.dt.bfloat16
i32 = mybir.dt.int32
AF = mybir.ActivationFunctionType
ALU = mybir.AluOpType
AX = mybir.AxisListType

# Problem constants
B_LOC = 2          # batches per core
D = 512
N = 128            # nodes
S = 14             # seq
L = 12
H = 8
DH = 64            # head dim
TOK = N * S        # 1792 tokens per batch, (s, n) order
C = 4              # 128-chunks of D
NCORES = 8

QUADS = [(0, 4), (4, 8), (8, 12), (12, 14)]  # s-blocks


def emit_kernel(ctx: ExitStack, tc: "tile.TileContext", io: dict):
    nc = tc.nc

    q_d, k_d, v_d, ab_d = io["q"], io["k"], io["v"], io["ab"]
    out_d = io["out"]

    # ---------------- pools ----------------
    wpool = ctx.enter_context(tc.tile_pool(name="wpool", bufs=1))
    tbp = ctx.enter_context(tc.tile_pool(name="tbp", bufs=1))
    bsbp = ctx.enter_context(tc.tile_pool(name="bsbp", bufs=2))
    abp = ctx.enter_context(tc.tile_pool(name="abp", bufs=1))
    xin = ctx.enter_context(tc.tile_pool(name="xin", bufs=8))
    qkh = ctx.enter_context(tc.tile_pool(name="qkh", bufs=1))
    vhp = ctx.enter_context(tc.tile_pool(name="vhp", bufs=2))
    ebp = ctx.enter_context(tc.tile_pool(name="ebp", bufs=3))
    ytp = ctx.enter_context(tc.tile_pool(name="ytp", bufs=4))
    vpp = ctx.enter_context(tc.tile_pool(name="vpp", bufs=2))
    zrp = ctx.enter_context(tc.tile_pool(name="zrp", bufs=1))
    osbp = ctx.enter_context(tc.tile_pool(name="osbp", bufs=2))

    pb = ctx.enter_context(tc.tile_pool(name="pb", bufs=2, space="PSUM"))
    pq = ctx.enter_context(tc.tile_pool(name="pq", bufs=2, space="PSUM"))
    scp = ctx.enter_context(tc.tile_pool(name="scp", bufs=2, space="PSUM"))

    # ---------------- weights (once) ----------------
    wq, wk, wv, wo = [], [], [], []
    for c in range(C):
        for dst, nm in ((wq, "wqT"), (wk, "wkT"), (wv, "wvT"), (wo, "woT")):
            t = wpool.tile([128, D], bf16, name=f"{nm}{c}", tag=f"{nm}{c}")
            nc.scalar.dma_start(t[:], io[nm][c * 128:(c + 1) * 128, :])
            dst.append(t)

    wdk2 = wpool.tile([96, 128], bf16, name="wdk2", tag="wdk2")
    nc.scalar.dma_start(wdk2[:], io["wdk2"][:])

    bk_t = wpool.tile([128, C], f32, name="bk_t", tag="bk_t")
    for c in range(C):
        nc.scalar.dma_start(bk_t[:, c:c + 1],
                            io["bk"][c * 128:(c + 1) * 128].unsqueeze(1))

    ones_b = wpool.tile([1, 128], bf16, name="ones_b", tag="ones_b")
    nc.vector.memset(ones_b[:], 1.0)
    bo_st = wpool.tile([1, D], bf16, name="bo_st", tag="bo_st")
    nc.gpsimd.dma_start(bo_st[:], io["bo"].unsqueeze(0))
    bv_t = wpool.tile([128, D], bf16, name="bv_t", tag="bv_t")
    nc.gpsimd.dma_start(bv_t[:], io["bv2"][:])

    # ---------------- shared bias tile (both batches) ----------------
    # tb free layout: (b2:2, s:16, hq:4, h2:2, ch:16, nq:4, n2:2), so the
    # per-(b,hq,h2,s) bias slice [128, n:128] is fully contiguous (the
    # stream-transpose output AP scatters (b2,s) outward; the n2 pair
    # rides inside each int32 element)
    tb = tbp.tile([128, 16 * 2048], bf16, name="tb", tag="tb")
    tb32v = tb[:].bitcast(i32).rearrange(
        "p (b2 s hq h2 ch nq) -> p ch hq h2 nq (b2 s)",
        b2=2, s=16, hq=4, h2=2, ch=16, nq=4)
    tbv = tb[:].rearrange(
        "p (b2 s hq h2 r) -> p b2 hq h2 s r",
        b2=2, s=16, hq=4, h2=2)

    def kron_gen():
        """ebias = exp(Wd-projected attn_bias), transposed to m-partition
        layout, both batches at once."""
        for quar in range(4):
            for half in range(2):
                abt = abp.tile([96, 4096], bf16, tag="abt", name="abt")
                nc.sync.dma_start(abt[:], ab_d[quar, half])
                abtv = abt[:].rearrange(
                    "p (c2 hq r) -> p c2 hq r", c2=2, hq=4)
                for chl in range(2):
                    ch = quar * 4 + half * 2 + chl
                    bsb = bsbp.tile([128, 2048], bf16, tag="bsb", name="bsb")
                    for t in range(2):
                        pbt = pb.tile([128, 1024], f32, tag="pb", name="pbt")
                        for hq2 in range(2):
                            hq = t * 2 + hq2
                            nc.tensor.matmul(
                                pbt[:, hq2 * 512:(hq2 + 1) * 512],
                                lhsT=wdk2[:],
                                rhs=abtv[:, chl, hq, :],
                                start=True, stop=True)
                        nc.scalar.activation(
                            bsb[:, t * 1024:(t + 1) * 1024], pbt[:], AF.Exp)
                        yield "kron"
                    nc.vector.transpose(tb32v[:, ch], bsb[:].bitcast(i32))
                    yield "kron"

    # ---------------- per-batch body ----------------
    def batch_work(b):
        def load_x(src_d):
            xs = []
            for ci in range(C):
                x_c = xin.tile([128, TOK], bf16, tag="xin", name=f"x{ci}")
                nc.gpsimd.dma_start(
                    x_c[:],
                    src_d[b, ci * 128:(ci + 1) * 128].rearrange(
                        "p s n -> p (s n)"))
                xs.append(x_c)
            return xs

        def p1_co(xs, wts, co, dst_list, with_bias, tg):
            # per-name tag, bufs=1: b1's gen reuses exactly b0's slot,
            # which frees when b0's P3 hp=co scores have read it
            h_c = qkh.tile([128, TOK], bf16, tag=f"h{tg}{co}", name=f"h{tg}{co}")
            for tbk in range(4):
                ps = pq.tile([128, 512], f32, tag="pq", name="ps_qk")
                for ci in range(C):
                    nc.tensor.matmul(
                        ps[:, :448],
                        lhsT=wts[ci][:, co * 128:(co + 1) * 128],
                        rhs=xs[ci][:, tbk * 448:(tbk + 1) * 448],
                        start=(ci == 0), stop=(ci == C - 1))
                if with_bias:
                    nc.scalar.activation(h_c[:, tbk * 448:(tbk + 1) * 448],
                                         ps[:, :448], AF.Identity,
                                         bias=bk_t[:, co:co + 1], scale=1.0)
                else:
                    # Q copies on DVE to balance ACT (kron exp runs there)
                    nc.vector.tensor_copy(h_c[:, tbk * 448:(tbk + 1) * 448],
                                          ps[:, :448])
            dst_list.append(h_c)

        # ---- P1 ----
        qh, kh = [], []
        yield "p1"
        xq = load_x(q_d)
        xk = load_x(k_d)
        for co in range(C):
            # gate marker: for b1 the driver holds here until b0's P3
            # hp=co scores are emitted (qkh slot release order)
            yield f"p1g{co}"
            p1_co(xq, wq, co, qh, False, "q")
            p1_co(xk, wk, co, kh, True, "k")
        yield "p1"
        xv = load_x(v_d)
        vh = vhp.tile([128, S * D], bf16, tag="vh", name="vh")
        for s in range(S):
            ps = pq.tile([128, D], f32, tag="pq", name="ps_v")
            for ci in range(C):
                nc.tensor.matmul(
                    ps[:],
                    lhsT=xv[ci][:, s * N:(s + 1) * N],
                    rhs=wv[ci][:],
                    start=(ci == 0), stop=(ci == C - 1))
            nc.vector.tensor_add(vh[:, s * D:(s + 1) * D], ps[:], bv_t[:])
            if s % 2:
                yield "p1"

        # ---- P3 ----
        z_t = zrp.tile([128, 128], f32, tag="z", name="z_t")
        rt_t = zrp.tile([128, 128], f32, tag="r", name="rt_t")
        zv = z_t[:].rearrange("p (h s) -> p h s", h=H)
        rv = rt_t[:].rearrange("p (h s) -> p h s", h=H)
        rvT = rt_t[:].rearrange("p (h s) -> p s h", h=H)
        vhv = vh[:].rearrange("p (s d) -> p s d", s=S)
        yt = [ytp.tile([128, TOK], bf16, tag="ytp", name=f"yt{c}")
              for c in range(C)]

        pend = [None]  # deferred AV work (one-quad software pipeline)

        def do_av(hp, qi, vpv, ex):
            s0, s1 = QUADS[qi]
            ls = s1 - s0
            av = pb.tile([128, 512], f32, tag="pb", name="av_t")
            for si in range(ls):
                for h2 in range(2):
                    nc.tensor.matmul(
                        av[h2 * DH:(h2 + 1) * DH, si * 128:(si + 1) * 128],
                        lhsT=vpv[:, si, h2, :],
                        rhs=ex[h2][:, si * 128:(si + 1) * 128],
                        start=True, stop=True)
            nc.scalar.copy(yt[hp][:, s0 * 128:s1 * 128], av[:, :ls * 128])

        for hp in range(C):
            for qi, (s0, s1) in enumerate(QUADS):
                ls = s1 - s0
                # scores for this quad
                sct = [scp.tile([128, 512], f32, tag="sc", name=f"sc{h2}")
                       for h2 in range(2)]
                for si in range(ls):
                    s = s0 + si
                    for h2 in range(2):
                        hb = h2 * DH
                        nc.tensor.matmul(
                            sct[h2][:, si * 128:(si + 1) * 128],
                            lhsT=kh[hp][hb:hb + DH, s * N:(s + 1) * N],
                            rhs=qh[hp][hb:hb + DH, s * N:(s + 1) * N],
                            start=True, stop=True)
                yield "p3"
                # deferred AV of previous quad (PE work lands after the
                # interleaved P1 units gave the chain time to finish)
                if pend[0] is not None:
                    do_av(*pend[0])
                    pend[0] = None
                # exp(scores) from psum; bias-mul with accum -> z
                ex = [ebp.tile([128, 512], bf16, tag="eb", name=f"eb{h2}")
                      for h2 in range(2)]
                for h2 in range(2):
                    nc.scalar.activation(ex[h2][:, :ls * 128],
                                         sct[h2][:, :ls * 128], AF.Exp)
                for h2 in range(2):
                    h = hp * 2 + h2
                    for si in range(ls):
                        s = s0 + si
                        esl = ex[h2][:, si * 128:(si + 1) * 128]
                        nc.vector.scalar_tensor_tensor(
                            esl, esl, 1.0, tbv[:, b, hp, h2, s],
                            op0=ALU.mult, op1=ALU.mult,
                            accum_out=zv[:, h, s:s + 1])
                nc.vector.reciprocal(rv[:, 2 * hp:2 * hp + 2, s0:s1],
                                     zv[:, 2 * hp:2 * hp + 2, s0:s1])
                # vp = vh * (1/z), broadcast over d
                vpt = vpp.tile([128, 512], bf16, tag="vp", name="vp_t")
                vpv = vpt[:, :ls * 128].rearrange(
                    "p (s h2 dd) -> p s h2 dd", s=ls, h2=2)
                src = vhv[:, s0:s1, hp * 128:(hp + 1) * 128].rearrange(
                    "p s (h2 dd) -> p s h2 dd", h2=2)
                rtb = rvT[:, s0:s1, 2 * hp:2 * hp + 2].unsqueeze(
                    3).broadcast_to([128, ls, 2, DH])
                nc.gpsimd.tensor_mul(vpv, src, rtb)
                pend[0] = (hp, qi, vpv, ex)
                yield "p3"
        do_av(*pend[0])
        pend[0] = None

        # ---- P4 ----
        for s in range(S):
            yield "p4"
            ps = pq.tile([128, D], f32, tag="pq", name="ps_o")
            for ci in range(C):
                nc.tensor.matmul(
                    ps[:],
                    lhsT=yt[ci][:, s * 128:(s + 1) * 128],
                    rhs=wo[ci][:],
                    start=(ci == 0), stop=False)
            nc.tensor.matmul(ps[:], lhsT=ones_b[:], rhs=bo_st[:],
                             start=False, stop=True)
            osb = osbp.tile([128, D], bf16, tag="osb", name="osb")
            nc.scalar.copy(osb[:], ps[:])
            eng = nc.sync if s % 2 == 0 else nc.scalar
            eng.dma_start(out_d[b, s], osb[:])

    # ---------------- driver ----------------
    # kron || P1(b0); kron fully drained before P3(b0)'s first stt hits
    # the DVE queue (its reads of tb must queue after all transposes);
    # then P3(b0) || P1(b1) with b1's QK-co_k gated on b0's hp_k scores
    # (qkh slot release order keeps the engine queues acyclic);
    # P3(b1)/P4(b1) emitted only after g0 is fully emitted.
    gk, g0, g1 = kron_gen(), batch_work(0), batch_work(1)
    t0 = next(g0)
    t1 = None
    g1_started = False
    kron_alive = True
    p3_cnt = 0

    def is_p1(t):
        return t is not None and t.startswith("p1")

    while t0 is not None or t1 is not None:
        if t0 is not None:
            t0 = next(g0, None)
            if t0 == "p3":
                p3_cnt += 1
        if kron_alive:
            # ~4 kron units per P1(b0) unit; full drain once g0 leaves P1
            n = 4 if is_p1(t0) else 10 ** 9
            for _ in range(n):
                if next(gk, None) is None:
                    kron_alive = False
                    break
        if not g1_started:
            if t0 is None or not is_p1(t0):
                g1_started = True
                t1 = next(g1, None)
        elif t1 is not None:
            if t0 is None:
                # g0 fully emitted: drain g1
                while t1 is not None:
                    t1 = next(g1, None)
            elif is_p1(t1):
                # advance g1 through its P1 only; hold at a gate marker
                # p1g{k} until b0's hp_k scores are emitted (8k+7 p3
                # yields); g1's P3 waits for g0 to finish entirely
                for _ in range(2):
                    if t1.startswith("p1g"):
                        k = int(t1[3:])
                        if p3_cnt < 8 * k + 7:
                            break
                    t1 = next(g1, None)
                    if t1 is None or not is_p1(t1):
                        break


def build_nc():
    nc = bacc.Bacc("TRN2", target_bir_lowering=False, debug=False,
                   num_devices=NCORES)
    io = {}
    # s-major tokens: [b, d, s, n]
    io["q"] = nc.dram_tensor("q", [B_LOC, D, S, N], bf16, kind="ExternalInput").ap()
    io["k"] = nc.dram_tensor("k", [B_LOC, D, S, N], bf16, kind="ExternalInput").ap()
    io["v"] = nc.dram_tensor("v", [B_LOC, D, S, N], bf16, kind="ExternalInput").ap()
    # ab host layout: [quar, half, (j b2 l)=96, (chl2 hq h2 nq mi n2)=4096]
    io["ab"] = nc.dram_tensor("ab", [4, 2, 96, 4096], bf16,
                              kind="ExternalInput").ap()
    for nm in ("wqT", "wkT", "wvT", "woT"):
        io[nm] = nc.dram_tensor(nm, [D, D], bf16, kind="ExternalInput").ap()
    io["bk"] = nc.dram_tensor("bk", [D], f32, kind="ExternalInput").ap()
    io["bv2"] = nc.dram_tensor("bv2", [128, D], bf16, kind="ExternalInput").ap()
    io["bo"] = nc.dram_tensor("bo", [D], bf16, kind="ExternalInput").ap()
    io["wdk2"] = nc.dram_tensor("wdk2", [96, 128], bf16, kind="ExternalInput").ap()
    io["out"] = nc.dram_tensor("out", [B_LOC, S, N, D], bf16,
                               kind="ExternalOutput").ap()

    with tile.TileContext(nc) as tc:
        with ExitStack() as ctx:
            emit_kernel(ctx, tc, io)
    nc.compile()
    return nc


def host_prep(Wq, bq, Wk, bk, Wv, bv, Wd, bd, Wo, bo):
    """Pre-transpose weights to bf16; fold qk scale into Wq; drop bq/bd
    (constant along the softmax axis); build the two-batch kron weight."""
    import ml_dtypes
    b16 = ml_dtypes.bfloat16
    scale = (D // H) ** -0.5
    prep = {
        "wqT": np.ascontiguousarray((Wq * scale).T).astype(b16),
        "wkT": np.ascontiguousarray(Wk.T).astype(b16),
        "wvT": np.ascontiguousarray(Wv.T).astype(b16),
        "woT": np.ascontiguousarray(Wo.T).astype(b16),
        "bk": np.asarray(bk, np.float32),
        "bv2": np.ascontiguousarray(
            np.broadcast_to(np.asarray(bv, np.float32), (128, D))).astype(b16),
        "bo": np.asarray(bo).astype(b16),
    }
    wdk2 = np.zeros((96, 128), np.float32)
    WdT = np.asarray(Wd).T  # [L, S]
    for j in range(4):
        for b2 in range(2):
            r0, c0 = j * 24 + b2 * 12, j * 32 + b2 * 16
            wdk2[r0:r0 + L, c0:c0 + S] = WdT
    prep["wdk2"] = wdk2.astype(b16)
    return prep


_NC_CACHE = None


def run(q, k, v, attn_bias, Wq, bq, Wk, bk, Wv, bv, Wd, bd, Wo, bo,
        trace=False, **trace_kwargs):
    global _NC_CACHE
    import ml_dtypes
    from concourse.bass_utils import run_bass_kernel_spmd

    b16 = ml_dtypes.bfloat16
    if _NC_CACHE is None:
        _NC_CACHE = build_nc()
    nc = _NC_CACHE

    prep = host_prep(Wq, bq, Wk, bk, Wv, bv, Wd, bd, Wo, bo)
    # [B, D, N, S] -> [B, D, S, N]
    q = np.asarray(q).transpose(0, 1, 3, 2).astype(b16)
    k = np.asarray(k).transpose(0, 1, 3, 2).astype(b16)
    v = np.asarray(v).transpose(0, 1, 3, 2).astype(b16)
    ab = np.asarray(attn_bias)
    B = ab.shape[0]

    in_maps = []
    for i in range(NCORES):
        sl = slice(i * B_LOC, (i + 1) * B_LOC)
        # per-core ab: [b2, l, n=(quar half chl2 nq n2), m=(j mi),
        # h=(hq h2)] -> [quar, half, j, b2, l, chl2, hq, h2, nq, mi, n2]
        abc = ab[sl].reshape(2, L, 4, 2, 2, 4, 2, 4, 32, 4, 2)
        abc = abc.transpose(2, 3, 7, 0, 1, 4, 9, 10, 5, 8, 6)
        abc = np.ascontiguousarray(abc).astype(b16).reshape(4, 2, 96, 4096)
        in_maps.append({
            "q": np.ascontiguousarray(q[sl]),
            "k": np.ascontiguousarray(k[sl]),
            "v": np.ascontiguousarray(v[sl]),
            "ab": abc,
            **prep,
        })
    res = run_bass_kernel_spmd(nc, in_maps, list(range(NCORES)), trace=trace,
                               **trace_kwargs)
    out = np.concatenate(
        [res.results[i]["out"].astype(np.float32) for i in range(NCORES)],
        axis=0)
    return out, res


def kernel(**inputs):
    return run(**inputs)[0]
